# revision 17
# baseline (speedup 1.0000x reference)
"""Local cross-attention (kNN gather) Trainium2 Bass kernel — v3.

Data-parallel over 40000 query points across 8 NeuronCores.

The kNN gather uses the Q7 batched DMA-gather ucode (InstDMAGatherAnt):
one instruction gathers all 4096 rows of a 128-query tile (~1us fixed
SWDGE cost paid once per tile instead of per 128 rows). Its indices are
int16, so each core's 40 tiles are split into 4 groups of 10; per group
the host deduplicates the referenced keys (<= 32768 by construction:
10*128*32 = 40960 refs, ~29.7k unique) and builds a compacted key table
+ int16 indices. Each group has its own projected KV table in DRAM.

Phase A (per group): project the compacted keys with lhsT = keysTc
block so PSUM comes out key-major; rhs = [Wk | Wv'] fused fp16 -> table
rows [K row | V' row] (512B fp16), no transposes. PSUM evictions go to
ACT/gpsimd (3:1), keeping DVE free for phase B.

Phase B (per tile): batched gather + fp16 attention on DVE with all
innermost strides unit (2x packed mode); no softmax max-subtraction
(scores provably bounded for this model family).

Cross-group overlap: emission is A(0) | barrier | B(0) interleaved with
A(1) | barrier | B(1) with A(2) | ... — the strict barrier before B(g)
orders it after A(g)'s table writes, while A(g+1) (emitted after that
barrier) only waits on the previous barrier and so overlaps B(g).

Mathematical simplifications (exact): bq, bk shift all scores of a
(query, head) uniformly -> softmax invariant -> dropped. bv adds a
constant to attended values -> folded host-side into bo' = bo + bv@Wo.
Wv columns are permuted host-side to (hd-major, head-minor) order and
Wo rows permuted to match, so the V-weighting broadcast has unit
innermost stride on the DVE.
"""

import numpy as np

N1, N2, D, H, K = 40000, 60000, 128, 8, 32
HD = D // H
SCALE = HD ** -0.5
NCORES = 8
N1C = N1 // NCORES          # 5000 queries per core
QT = 128                    # queries per tile
N1P = 5120                  # padded queries per core -> 40 tiles
NT = N1P // QT
GROUPS = 4
GT = NT // GROUPS           # 10 tiles per group
GQ = GT * QT                # 1280 queries per group
GCAP = 32768                # table rows per group (int16 index space)
CH = 2048                   # phase-A chunk of keys (16 blocks of 128)
NCH = GCAP // CH            # 32 chunks per group
E = 2 * D                   # interleaved KV row length (256 fp16 = 512B)
ISB = 256                   # idx tile free dim: 4096 idxs wrapped in 16

_PROG = None


def _build():
    import concourse.bass as bass
    import concourse.tile as tile
    from concourse import bacc, mybir, library_config
    from concourse.masks import make_identity
    from contextlib import ExitStack

    f32 = mybir.dt.float32
    f16 = mybir.dt.float16
    i16 = mybir.dt.int16
    AX = mybir.AxisListType
    OP = mybir.AluOpType
    AF = mybir.ActivationFunctionType

    nc = bacc.Bacc("TRN2", target_bir_lowering=False, debug=False,
                   enable_asserts=True, num_devices=1,
                   num_swdge_queues=4)

    qT = nc.dram_tensor("qT", [D, N1P], f16, kind="ExternalInput").ap()
    keysTc = nc.dram_tensor("keysTc", [D, GROUPS * GCAP], f16,
                            kind="ExternalInput").ap()
    knn16 = nc.dram_tensor("knn16", [N1P, ISB], i16,
                           kind="ExternalInput").ap()
    wq = nc.dram_tensor("wq", [D, D], f16, kind="ExternalInput").ap()
    wkv = nc.dram_tensor("wkv", [D, E], f16, kind="ExternalInput").ap()
    wo = nc.dram_tensor("wo", [D, D], f16, kind="ExternalInput").ap()
    bo2 = nc.dram_tensor("bo2", [D, 1], f32, kind="ExternalInput").ap()
    outT = nc.dram_tensor("outT", [D, N1P], f32, kind="ExternalOutput").ap()
    tables = nc.dram_tensor("kv_tables", [GROUPS * GCAP, E], f16,
                            kind="Internal").ap()

    with nc.allow_low_precision("fp16 attention; 2e-2 rel tolerance"), \
         tile.TileContext(nc) as tc:
        with ExitStack() as st:
            cp = st.enter_context(tc.tile_pool(name="const", bufs=1))
            ident = cp.tile([128, 128], f16)
            make_identity(nc, ident[:])
            wq_s = cp.tile([D, D], f16, tag="wq")
            wkv_s = cp.tile([D, E], f16, tag="wkv")
            wo_s = cp.tile([D, D], f16, tag="wo")
            bo_s = cp.tile([D, 1], f32, tag="bo")
            for sb, dr in ((wq_s, wq), (wkv_s, wkv), (wo_s, wo), (bo_s, bo2)):
                nc.sync.dma_start(sb[:], dr)
            qT_s = cp.tile([D, N1P], f16, tag="qTs")
            nc.sync.dma_start(qT_s[:], qT)

            ap_ = st.enter_context(tc.tile_pool(name="pa_sb", bufs=3))
            op_ = st.enter_context(tc.tile_pool(name="pa_out", bufs=3))
            psA = st.enter_context(
                tc.tile_pool(name="pa_ps", bufs=2, space="PSUM"))
            ixp = st.enter_context(tc.tile_pool(name="pb_ix", bufs=3))
            kvp = st.enter_context(tc.tile_pool(name="pb_kv", bufs=3))
            ppp = st.enter_context(tc.tile_pool(name="pb_prod", bufs=3))
            ssp = st.enter_context(tc.tile_pool(name="pb_small", bufs=3))
            psp = st.enter_context(
                tc.tile_pool(name="pb_ps", bufs=2, space="PSUM"))

            def emit_a_chunk(g, c):
                # project compacted keys [g, c*CH:(c+1)*CH] -> table rows
                kc = ap_.tile([128, CH], f16, tag="kc")
                nc.sync.dma_start(
                    kc[:], keysTc[:, bass.ts(g * NCH + c, CH)])
                kvrows = op_.tile([128, (CH // 128) * E], f16, tag="kvr")
                for hb in range(CH // 256):
                    ps = psA.tile([128, 512], f32, tag="ps")
                    nc.tensor.matmul(ps[:, 0:E],
                                     lhsT=kc[:, bass.ts(2 * hb, 128)],
                                     rhs=wkv_s[:], start=True, stop=True)
                    nc.tensor.matmul(ps[:, E:2 * E],
                                     lhsT=kc[:, bass.ts(2 * hb + 1, 128)],
                                     rhs=wkv_s[:], start=True, stop=True)
                    dst = kvrows[:, hb * 2 * E:(hb + 1) * 2 * E]
                    nc.scalar.activation(dst, ps[:], AF.Copy)
                nc.sync.dma_start(
                    tables[(g * NCH + c) * CH:(g * NCH + c + 1) * CH, :]
                    .rearrange("(p b) e -> p b e", p=128),
                    kvrows[:].rearrange("p (b e) -> p b e", e=E))

            def emit_b_tile(g, t):
                i = g * GT + t
                idx = ixp.tile([128, ISB], i16, tag="idx")
                nc.sync.dma_start(idx[:], knn16[bass.ts(i, 128), :])
                kv = kvp.tile([128, K * E], f16, tag="kv")
                kv3 = kv[:].rearrange("p (k e) -> p k e", e=E)
                # batched gather: 4x1024 rows of 512B from this group's
                # table (>=2048 idxs per instruction wedges the SWDGE
                # ring); lands as [q, k, e], 8 k-slots per call
                for m in range(4):
                    nc.gpsimd.dma_gather(
                        kv3[:, m * (K // 4):(m + 1) * (K // 4), :],
                        tables[g * GCAP:(g + 1) * GCAP, :],
                        idx[:, m * (ISB // 4):(m + 1) * (ISB // 4)],
                        K * QT // 4, K * QT // 4, E, queue_num=m)

                psQ = psp.tile([128, 128], f32, tag="psQ")
                nc.tensor.matmul(psQ[:], lhsT=qT_s[:, bass.ts(i, QT)],
                                 rhs=wq_s[:], start=True, stop=True)
                qrow = ssp.tile([128, D], f16, tag="qrow")
                nc.scalar.activation(qrow[:], psQ[:], AF.Copy, scale=SCALE)

                # scores: prod[q, k, d] = K_g[q,k,d] * (SCALE*Q)[q,d]
                prod = ppp.tile([128, K * D], f16, tag="prod")
                nc.vector.tensor_tensor(
                    out=prod[:].rearrange("p (k d) -> p k d", d=D),
                    in0=kv3[:, :, 0:D],
                    in1=qrow[:].unsqueeze(1).broadcast_to([128, K, D]),
                    op=OP.mult)
                sc = ssp.tile([128, K * H], f16, tag="sc")
                nc.vector.tensor_reduce(
                    out=sc[:],
                    in_=prod[:].rearrange("p (s d) -> p s d", d=HD),
                    axis=AX.X, op=OP.add)
                # softmax over k ([q, (k,h)]); scores bounded -> no max
                ee = ssp.tile([128, K * H], f16, tag="ee")
                nc.scalar.activation(ee[:], sc[:], AF.Exp)
                den = ssp.tile([128, H], f16, tag="den")
                nc.vector.tensor_reduce(
                    out=den[:],
                    in_=ee[:].rearrange("p (k h) -> p h k", h=H),
                    axis=AX.X, op=OP.add)
                rden = ssp.tile([128, H], f16, tag="rden")
                nc.vector.reciprocal(rden[:], den[:])

                # weighted V (V stored hd-major): prod2[q,k,hd,h]
                prod2 = ppp.tile([128, K * D], f16, tag="prod2")
                nc.vector.tensor_tensor(
                    out=prod2[:].rearrange("p (k hd h) -> p k hd h",
                                           h=H, hd=HD),
                    in0=kv3[:, :, D:E].rearrange(
                        "p k (hd h) -> p k hd h", h=H),
                    in1=ee[:].rearrange("p (k h) -> p k h", h=H)
                        .unsqueeze(2).broadcast_to([128, K, HD, H]),
                    op=OP.mult)
                # tree-reduce over k: 5 contiguous halving adds
                v3 = prod2[:].rearrange("p (k e) -> p k e", e=D)
                half = K // 2
                while half >= 1:
                    nc.vector.tensor_tensor(
                        out=v3[:, 0:half, :], in0=v3[:, 0:half, :],
                        in1=v3[:, half:2 * half, :], op=OP.add)
                    half //= 2
                # normalize: attn[q, (hd h)] = att * (1/den)[h]
                attn = ssp.tile([128, D], f16, tag="attn")
                nc.vector.tensor_tensor(
                    out=attn[:].rearrange("p (hd h) -> p hd h", h=H),
                    in0=v3[:, 0, :].rearrange("p (hd h) -> p hd h", h=H),
                    in1=rden[:].unsqueeze(1).broadcast_to([128, HD, H]),
                    op=OP.mult)

                # output projection: outT[:, tile] = Wo'^T @ attn^T + bo'
                psAT = psp.tile([128, 128], f16, tag="psAT")
                nc.tensor.transpose(psAT[:], attn[:], ident[:])
                cAT = ssp.tile([128, 128], f16, tag="cAT")
                nc.scalar.activation(cAT[:], psAT[:], AF.Copy)
                psO = psp.tile([128, 128], f32, tag="psO")
                nc.tensor.matmul(psO[:], lhsT=wo_s[:], rhs=cAT[:],
                                 start=True, stop=True)
                oT = ssp.tile([128, 128], f32, tag="oT")
                nc.scalar.activation(oT[:], psO[:], AF.Identity,
                                     bias=bo_s[:, :])
                nc.scalar.dma_start(outT[:, bass.ts(i, QT)], oT[:])

            # group 0 table build, then: barrier | B(g) interleaved with
            # A(g+1) | barrier | ... — A(g+1) only waits on the previous
            # barrier, so it overlaps B(g).
            for c in range(NCH):
                emit_a_chunk(0, c)
            for g in range(GROUPS):
                tc.strict_bb_all_engine_barrier()
                for t in range(GT):
                    if g + 1 < GROUPS:
                        for c in range(NCH * t // GT, NCH * (t + 1) // GT):
                            emit_a_chunk(g + 1, c)
                    emit_b_tile(g, t)

    nc.compile()
    return nc


def _get_prog():
    global _PROG
    if _PROG is None:
        _PROG = _build()
    return _PROG


def _make_in_maps(inputs):
    qf = np.asarray(inputs["query_features"], np.float32)
    kf = np.asarray(inputs["key_features"], np.float32)
    ki = np.asarray(inputs["knn_indices"])
    Wq = np.asarray(inputs["Wq"], np.float32)
    Wk = np.asarray(inputs["Wk"], np.float32)
    Wv = np.asarray(inputs["Wv"], np.float32)
    Wo = np.asarray(inputs["Wo"], np.float32)
    bv = np.asarray(inputs["bv"], np.float32)
    bo = np.asarray(inputs["bo"], np.float32)

    # permutation: e' = hd*H + h  <->  e = h*HD + hd
    perm = np.arange(D).reshape(H, HD).T.reshape(-1)

    keysT16 = np.ascontiguousarray(kf.T).astype(np.float16)  # [D, N2]
    wq16 = Wq.astype(np.float16)
    wkv16 = np.concatenate([Wk, Wv[:, perm]], axis=1).astype(np.float16)
    wo16 = np.ascontiguousarray(Wo[perm, :]).astype(np.float16)
    bo2 = (bo + bv @ Wo).astype(np.float32).reshape(D, 1)

    in_maps = []
    for c in range(NCORES):
        qTc = np.zeros((D, N1P), np.float16)
        qTc[:, :N1C] = qf[c * N1C:(c + 1) * N1C].T.astype(np.float16)
        knnc = np.zeros((N1P, K), np.int32)
        knnc[:N1C] = ki[c * N1C:(c + 1) * N1C].astype(np.int32)

        keysTcg = np.zeros((D, GROUPS * GCAP), np.float16)
        k16 = np.empty((N1P, ISB), np.int16)
        for g in range(GROUPS):
            rows = knnc[g * GQ:(g + 1) * GQ]              # [GQ, K]
            uniq, inv = np.unique(rows, return_inverse=True)
            assert len(uniq) <= GCAP, len(uniq)
            # table rows are written p-major within each chunk (so each
            # partition writes 16 consecutive rows): column x = c*CH +
            # b*128 + p lands at row c*CH + p*(CH//128) + b
            cc = inv // CH
            wi = inv % CH
            inv = cc * CH + (wi % 128) * (CH // 128) + wi // 128
            keysTcg[:, g * GCAP:g * GCAP + len(uniq)] = keysT16[:, uniq]
            invr = inv.reshape(GQ, K).astype(np.int16)
            for t in range(GT):
                tile_inv = invr[t * QT:(t + 1) * QT]       # [128, K]
                row = np.empty((16, ISB), np.int16)
                for m in range(4):
                    flat = tile_inv[:, m * (K // 4):(m + 1) * (K // 4)] \
                        .T.ravel()                          # j = kl*128+q
                    row[:, m * (ISB // 4):(m + 1) * (ISB // 4)] = \
                        flat.reshape(ISB // 4, 16).T
                k16[(g * GT + t) * QT:(g * GT + t + 1) * QT] = \
                    np.tile(row, (8, 1))
        in_maps.append({
            "qT": qTc, "keysTc": keysTcg, "knn16": k16,
            "wq": wq16, "wkv": wkv16, "wo": wo16, "bo2": bo2,
        })
    return in_maps


def kernel(query_features, key_features, knn_indices,
           Wq, bq, Wk, bk, Wv, bv, Wo, bo):
    from concourse import bass_utils

    nc = _get_prog()
    in_maps = _make_in_maps({
        "query_features": query_features, "key_features": key_features,
        "knn_indices": knn_indices, "Wq": Wq, "bq": bq, "Wk": Wk, "bk": bk,
        "Wv": Wv, "bv": bv, "Wo": Wo, "bo": bo,
    })

    res = bass_utils.run_bass_kernel_spmd(
        nc, in_maps, core_ids=list(range(NCORES)))

    out = np.empty((N1, D), np.float32)
    for c in range(NCORES):
        out[c * N1C:(c + 1) * N1C] = res.results[c]["outT"][:, :N1C].T
    return out
